# revision 2
# baseline (speedup 1.0000x reference)
"""Trainium2 Bass kernel for nn_DKAModule (dynamic-kernel attention).

Decomposition (per core, data-parallel over B*n = 8192 tokens -> 1024/core
with a 10-token halo), all matmuls in bf16 (1 cycle/row on the PE vs 2 for
fp32r), PSUM accumulation in fp32:

  x_projT = W_in @ x^T                       (PE, layout (d, t), bf16)
  per head h (d_h=128 partitions, window size k_h):
    S_r    = band-matrix matmuls over token windows (PE)   [dynamic conv]
    csv_r  = S_r * cbv_r                     (DVE; cbv = alpha*V[r,d]*c_r[n]
                                              precomputed on host, bf16)
    o_h    = sum_r csv_r                     (DVE add tree, or PE ident-diag)
           + static conv: head 7 on PE diag matmuls, heads 0-6 on DVE as
             tensor_scalar (4x bf16) + tensor_add (2x bf16) shift-MACs
  out     = o^T @ W_out^T                    (PE); b_out added on host

Act (scalar) engine handles every PSUM evacuation so DVE operands stay
all-SBUF bf16 (fast 2x/4x DVE modes).
"""
import sys
import types

import numpy as np
import ml_dtypes

BF = ml_dtypes.bfloat16

KS = [3, 3, 7, 7, 11, 11, 21, 21]
H, DM, DH, R, B, N = 8, 1024, 128, 4, 2, 4096
NC = 8
TPC = B * N // NC  # tokens per core
PAD = 10
TH = TPC + 2 * PAD  # 1044
S1C = TH // 3  # 348, stage-1 chunk width

PE_STATIC_HEADS = (7,)  # static conv + r-sum on PE (others: DVE)
HEAD_ORDER = (7, 6, 0, 1, 4, 2, 5, 3)

_MODULE_CACHE = {}


def _install_ntff_hook_shim():
    """This image's antenv lacks axon_hooks; provide it so profiling works."""
    if "antenv.axon_hooks" in sys.modules:
        return
    try:
        from trn_agent_boot.trn_boot import _ntff_profile_via_ctypes

        hook = _ntff_profile_via_ctypes("/opt/axon/libaxon_pjrt.so")
    except Exception:
        hook = None
    mod = types.ModuleType("antenv.axon_hooks")
    mod.get_axon_ntff_profile_hook = lambda: hook
    mod.set_axon_ntff_profile_hook = lambda h: None
    sys.modules["antenv.axon_hooks"] = mod


def _split_multi_waits(nc, mybir):
    """walrus codegen allows a single sync-wait per instruction; hoist
    extras onto a chain of single-wait NoOps on the same engine."""
    for f in nc.m.functions:
        for blk in f.blocks:
            new_insts = []
            for inst in blk.instructions:
                si = getattr(inst, "sync_info", None)
                ow = list(si.on_wait) if si and si.on_wait else []
                if len(ow) >= 2:
                    for i, w in enumerate(ow[:-1]):
                        new_insts.append(
                            mybir.InstNoOp(
                                name=f"{inst.name}-wn{i}",
                                ins=[],
                                outs=[],
                                engine=inst.engine,
                                sync_info=mybir.SyncInfo(on_wait=[w], on_update=[]),
                            )
                        )
                    inst.sync_info = mybir.SyncInfo(
                        on_wait=[ow[-1]],
                        on_update=list(si.on_update) if si.on_update else [],
                    )
                new_insts.append(inst)
            blk.instructions = new_insts


def _window_params(h):
    k = KS[h]
    p = k // 2
    W = 128 - 2 * p
    nw = -(-TPC // W)
    return k, p, W, nw


def _build_module():
    import concourse.bass as bass
    import concourse.tile as tile
    from concourse import mybir

    f32 = mybir.dt.float32
    bf16 = mybir.dt.bfloat16
    MUL = mybir.AluOpType.mult

    nc = bass.Bass(trn_type="TRN2")

    # ---- DRAM I/O ----
    xT_d = nc.dram_tensor("xT", [DM, TH], bf16, kind="ExternalInput")
    w_inT_d = nc.dram_tensor("w_inT", [DM, DM], bf16, kind="ExternalInput")
    w_outT_d = nc.dram_tensor("w_outT", [DM, DM], bf16, kind="ExternalInput")
    cbv_d = nc.dram_tensor("cbv", [128, H * R * TPC], bf16, kind="ExternalInput")
    band_d = []
    for h in range(H):
        k, p, W, nw = _window_params(h)
        band_d.append(
            nc.dram_tensor(f"band{h}", [128, R * W], bf16, kind="ExternalInput")
        )
    gd_d = {
        h: nc.dram_tensor(f"gdiag{h}", [DH, KS[h] * DH], bf16, kind="ExternalInput")
        for h in PE_STATIC_HEADS
    }
    gvec_d = nc.dram_tensor("gvec", [DH, H * 21], f32, kind="ExternalInput")
    b_in_d = nc.dram_tensor("b_in", [DM, 1], f32, kind="ExternalInput")
    ident_d = nc.dram_tensor("ident", [128, 128], bf16, kind="ExternalInput")
    out_d = nc.dram_tensor("out", [TPC, DM], bf16, kind="ExternalOutput")

    with tile.TileContext(nc) as tc:
        with tc.tile_pool(name="const", bufs=1) as pc:
            xp_sb = [pc.tile([DH, TH], bf16, name=f"xp{m}") for m in range(H)]
            o_sb = [pc.tile([DH, TPC], bf16, name=f"o{h}") for h in range(H)]
            ident_sb = pc.tile([128, 128], bf16, name="ident_sb")
            b_in_sb = pc.tile([128, H], f32, name="b_in_sb")
            gvec_sb = pc.tile([DH, H * 21], f32, name="gvec_sb")
            nc.gpsimd.dma_start(out=ident_sb, in_=ident_d[:, :])
            nc.gpsimd.dma_start(out=gvec_sb, in_=gvec_d[:, :])
            for m in range(H):
                nc.gpsimd.dma_start(
                    out=b_in_sb[:, m : m + 1], in_=b_in_d[m * 128 : (m + 1) * 128, :]
                )

            # ---------------- stage 1: x_projT ----------------
            with tc.tile_pool(name="s1", bufs=1) as p1, tc.tile_pool(
                name="ps1", bufs=4, space="PSUM"
            ) as pp1:
                w_sb = [p1.tile([128, DM], bf16, name=f"w_in{i}") for i in range(H)]
                xT_sb = [p1.tile([128, TH], bf16, name=f"xT{i}") for i in range(H)]
                # first m-group's operands first, then the bulk
                for i in range(H):
                    nc.sync.dma_start(
                        out=w_sb[i][:, 0:128],
                        in_=w_inT_d[i * 128 : (i + 1) * 128, 0:128],
                    )
                    nc.gpsimd.dma_start(
                        out=xT_sb[i][:, 0:S1C],
                        in_=xT_d[i * 128 : (i + 1) * 128, 0:S1C],
                    )
                for i in range(H):
                    nc.sync.dma_start(
                        out=w_sb[i][:, 128:DM],
                        in_=w_inT_d[i * 128 : (i + 1) * 128, 128:DM],
                    )
                    nc.sync.dma_start(
                        out=xT_sb[i][:, S1C:TH],
                        in_=xT_d[i * 128 : (i + 1) * 128, S1C:TH],
                    )

                for ci in range(3):
                    c0 = ci * S1C
                    for m in HEAD_ORDER:
                        ps = pp1.tile([128, S1C], f32, name="ps_xp", tag="ps_xp")
                        for i in range(H):
                            nc.tensor.matmul(
                                ps,
                                w_sb[i][:, m * 128 : (m + 1) * 128],
                                xT_sb[i][:, c0 : c0 + S1C],
                                start=(i == 0),
                                stop=(i == H - 1),
                            )
                        nc.scalar.activation(
                            out=xp_sb[m][:, c0 : c0 + S1C],
                            in_=ps,
                            func=mybir.ActivationFunctionType.Identity,
                            bias=b_in_sb[:, m : m + 1],
                            scale=1.0,
                        )

            # prefetch out-projection weights during stage 2
            p4ctx = tc.tile_pool(name="s4w", bufs=1)
            p4 = p4ctx.__enter__()
            wo_sb = []
            for i in range(H):
                wt = p4.tile([128, DM], bf16, name=f"w_out{i}")
                nc.gpsimd.dma_start(out=wt, in_=w_outT_d[i * 128 : (i + 1) * 128, :])
                wo_sb.append(wt)

            # ---------------- stage 2: per-head convs ----------------
            with tc.tile_pool(name="s2b", bufs=2) as p2b, tc.tile_pool(
                name="s2cbv", bufs=3
            ) as p2c, tc.tile_pool(name="s2csv", bufs=3) as p2v, tc.tile_pool(
                name="s2s", bufs=3
            ) as p2s, tc.tile_pool(name="s2x", bufs=4) as p2x, tc.tile_pool(
                name="s2acc", bufs=2
            ) as p2a, tc.tile_pool(name="s2t", bufs=3) as p2t, tc.tile_pool(
                name="ps2tp", bufs=2, space="PSUM"
            ) as pptp, tc.tile_pool(
                name="ps2s", bufs=2, space="PSUM"
            ) as ppss, tc.tile_pool(
                name="ps2o", bufs=2, space="PSUM"
            ) as ppso:
                cbv4 = cbv_d.rearrange("p (hh r t) -> p hh r t", hh=H, r=R)
                gd_sb = {}
                for h in PE_STATIC_HEADS:
                    g = pc.tile([DH, KS[h] * DH], bf16, name=f"gd{h}")
                    nc.sync.dma_start(out=g, in_=gd_d[h][:, :])
                    gd_sb[h] = g

                csv3_of = {}

                def stage2a(h):
                    k, p, W, nw = _window_params(h)
                    xph = xp_sb[h]
                    band_sb = p2b.tile([128, R * W], bf16, name=f"band{h}", tag="band")
                    nc.sync.dma_start(out=band_sb, in_=band_d[h][:, :])
                    band3 = band_sb.rearrange("p (r w) -> p r w", r=R)
                    cbv_sb = p2c.tile([DH, R * TPC], bf16, name=f"cbv{h}", tag="cbv")
                    nc.gpsimd.dma_start(
                        out=cbv_sb.rearrange("p (r t) -> p r t", r=R),
                        in_=cbv4[:, h, :, :],
                    )
                    cbv3 = cbv_sb.rearrange("p (r t) -> p r t", r=R)
                    csv_sb = p2v.tile([DH, R * TPC], bf16, name=f"csv{h}", tag="csv")
                    csv3 = csv_sb.rearrange("p (r t) -> p r t", r=R)
                    csv3_of[h] = csv3
                    for b in range(nw):
                        off = PAD - p + b * W
                        cnt = min(128, TH - off)
                        n_out = min(W, TPC - b * W)
                        tp = pptp.tile([128, 128], bf16, name="tp", tag="tp")
                        nc.tensor.transpose(
                            tp[:cnt, :], xph[:, off : off + cnt], ident_sb
                        )
                        xtd = p2x.tile([128, 128], bf16, name="xtd", tag="xtd")
                        nc.scalar.copy(xtd[:cnt, :], tp[:cnt, :])
                        ps_s = ppss.tile([128, R * W], f32, name="ps_s", tag="ps_s")
                        nc.tensor.matmul(
                            ps_s[:, : R * n_out],
                            xtd[:cnt, :],
                            band3[:cnt, :, :n_out],
                            start=True,
                            stop=True,
                        )
                        s_sb = p2s.tile([128, R * W], bf16, name="s_sb", tag="s_sb")
                        nc.scalar.copy(s_sb[:, : R * n_out], ps_s[:, : R * n_out])
                        nc.vector.tensor_mul(
                            csv3[:, :, b * W : b * W + n_out],
                            s_sb[:, : R * n_out].rearrange("p (r w) -> p r w", r=R),
                            cbv3[:, :, b * W : b * W + n_out],
                        )

                def stage2b(h):
                    k, p, W, nw = _window_params(h)
                    xph = xp_sb[h]
                    csv3 = csv3_of[h]
                    if h in PE_STATIC_HEADS:
                        for c0 in (0, 512):
                            ps_o = ppso.tile([128, 512], f32, name="ps_o", tag="ps_o")
                            nmm = R + k
                            idx = 0
                            for rr in range(R):
                                nc.tensor.matmul(
                                    ps_o,
                                    ident_sb,
                                    csv3[:, rr, c0 : c0 + 512],
                                    start=(idx == 0),
                                    stop=(idx == nmm - 1),
                                )
                                idx += 1
                            for j in range(k):
                                o0 = c0 + j - p + PAD
                                nc.tensor.matmul(
                                    ps_o,
                                    gd_sb[h][:, j * DH : (j + 1) * DH],
                                    xph[:, o0 : o0 + 512],
                                    start=(idx == 0),
                                    stop=(idx == nmm - 1),
                                )
                                idx += 1
                            nc.scalar.copy(o_sb[h][:, c0 : c0 + 512], ps_o)
                    else:
                        sacc = p2a.tile([DH, TPC], bf16, name="sacc", tag="sacc")
                        for j in range(k):
                            sh = PAD + j - p
                            gcol = gvec_sb[:, h * 21 + j : h * 21 + j + 1]
                            if j == 0:
                                nc.vector.tensor_scalar(
                                    out=sacc,
                                    in0=xph[:, sh : sh + TPC],
                                    scalar1=gcol,
                                    scalar2=None,
                                    op0=MUL,
                                )
                            else:
                                tmp = p2t.tile([DH, TPC], bf16, name="tmp", tag="tmp")
                                nc.vector.tensor_scalar(
                                    out=tmp,
                                    in0=xph[:, sh : sh + TPC],
                                    scalar1=gcol,
                                    scalar2=None,
                                    op0=MUL,
                                )
                                nc.vector.tensor_add(sacc, sacc, tmp)
                        t01 = p2t.tile([DH, TPC], bf16, name="t01", tag="tmp")
                        nc.vector.tensor_add(t01, csv3[:, 0, :], csv3[:, 1, :])
                        t23 = p2t.tile([DH, TPC], bf16, name="t23", tag="tmp")
                        nc.vector.tensor_add(t23, csv3[:, 2, :], csv3[:, 3, :])
                        nc.vector.tensor_add(t01, t01, t23)
                        nc.vector.tensor_add(o_sb[h], t01, sacc)

                for hi, h in enumerate(HEAD_ORDER):
                    stage2a(h)
                    if hi >= 1:
                        stage2b(HEAD_ORDER[hi - 1])
                stage2b(HEAD_ORDER[-1])

            # ---------------- stage 4: out projection ----------------
            with tc.tile_pool(name="s4o", bufs=3) as p4o, tc.tile_pool(
                name="ps4", bufs=4, space="PSUM"
            ) as pp4:
                for t in range(TPC // 128):
                    ot = p4o.tile([128, DM], bf16, name="out_sb", tag="out_sb")
                    for e0 in (0, 512):
                        ps = pp4.tile([128, 512], f32, name="ps_out", tag="ps_out")
                        for i in range(H):
                            nc.tensor.matmul(
                                ps,
                                o_sb[i][:, t * 128 : (t + 1) * 128],
                                wo_sb[i][:, e0 : e0 + 512],
                                start=(i == 0),
                                stop=(i == H - 1),
                            )
                        nc.scalar.copy(ot[:, e0 : e0 + 512], ps)
                        nc.gpsimd.dma_start(
                            out=out_d[t * 128 : (t + 1) * 128, e0 : e0 + 512],
                            in_=ot[:, e0 : e0 + 512],
                        )
            p4ctx.__exit__(None, None, None)

    _split_multi_waits(nc, mybir)
    return nc


def _host_prep(inputs):
    x = np.ascontiguousarray(np.asarray(inputs["x"], dtype=np.float32))
    W_in = np.asarray(inputs["W_in"], dtype=np.float32)
    b_in = np.asarray(inputs["b_in"], dtype=np.float32)
    W_out = np.asarray(inputs["W_out"], dtype=np.float32)
    b_out = np.asarray(inputs["b_out"], dtype=np.float32)
    Wc = np.asarray(inputs["Wc"], dtype=np.float32)
    A = np.asarray(inputs["A"], dtype=np.float32)
    V = np.asarray(inputs["V"], dtype=np.float32)
    base = np.asarray(inputs["base"], dtype=np.float32)
    alphas = np.asarray(inputs["alphas"], dtype=np.float32)

    alpha = 1.0 / (1.0 + np.exp(-alphas))

    W_inT = np.ascontiguousarray(W_in.T)
    W_outT = np.ascontiguousarray(W_out.T)
    # c = x_proj_head @ Wc[h] folded to raw x: x @ Wc_aug + c_bias (from b_in)
    Wc_aug = np.zeros((DM, H * R), dtype=np.float32)
    c_bias = np.zeros((H * R,), dtype=np.float32)
    for h in range(H):
        Wc_aug[:, R * h : R * h + R] = W_inT[:, h * DH : (h + 1) * DH] @ Wc[h]
        c_bias[R * h : R * h + R] = b_in[h * DH : (h + 1) * DH] @ Wc[h]

    prep = {
        "w_inT": W_inT.astype(BF),
        "w_outT": W_outT.astype(BF),
        "b_in": b_in.reshape(DM, 1).copy(),
        "ident": np.eye(128, dtype=BF),
    }

    for h in range(H):
        k, p, W, nw = _window_params(h)
        t_in = np.arange(128)[:, None]
        t_out = np.arange(W)[None, :]
        delta = t_in - t_out
        mask = (delta >= 0) & (delta < k)
        band = np.zeros((128, R, W), dtype=np.float32)
        dc = np.clip(delta, 0, k - 1)
        for rr in range(R):
            band[:, rr, :] = np.where(mask, A[h, rr][dc], 0.0)
        prep[f"band{h}"] = band.reshape(128, R * W).astype(BF)

    for h in PE_STATIC_HEADS:
        k = KS[h]
        gd = np.zeros((DH, k, DH), dtype=np.float32)
        g = (1.0 - alpha[h]) * base[h, :k]  # (k, DH)
        dd = np.arange(DH)
        gd[dd, :, dd] = g.T[dd]
        prep[f"gdiag{h}"] = gd.reshape(DH, k * DH).astype(BF)

    gvec = np.zeros((DH, H, 21), dtype=np.float32)
    for h in range(H):
        k = KS[h]
        gvec[:, h, :k] = ((1.0 - alpha[h]) * base[h, :k]).T
    prep["gvec"] = gvec.reshape(DH, H * 21).copy()

    # per-core transposed x slices with halo + zero padding, plus host-side
    # coefficient computation with alpha*V folded in:
    # cbv[d, h, r, t] = alpha[h] * V[h,r,d] * c[t, h, r]
    xT_slices = []
    cbv_slices = []
    per_b = NC // B
    for c in range(NC):
        bb = c // per_b
        s = (c % per_b) * TPC
        sl = np.zeros((TH, DM), dtype=np.float32)
        lo, hi = s - PAD, s + TPC + PAD
        clo, chi = max(lo, 0), min(hi, N)
        sl[clo - lo : chi - lo] = x[bb, clo:chi]
        xT_slices.append(np.ascontiguousarray(sl.T).astype(BF))
        cc = sl[PAD : PAD + TPC] @ Wc_aug + c_bias[None, :]  # (TPC, H*R)
        cc3 = cc.reshape(TPC, H, R)
        cbv = np.empty((128, H, R, TPC), dtype=np.float32)
        for h in range(H):
            for rr in range(R):
                cbv[:, h, rr, :] = alpha[h] * np.outer(V[h, rr], cc3[:, h, rr])
        cbv_slices.append(
            np.ascontiguousarray(cbv.reshape(128, H * R * TPC)).astype(BF)
        )
    return prep, xT_slices, cbv_slices, b_out


def _run(inputs, trace=False, **kwargs):
    _install_ntff_hook_shim()
    from concourse.bass_utils import run_bass_kernel_spmd

    if "mod" not in _MODULE_CACHE:
        _MODULE_CACHE["mod"] = _build_module()
    nc = _MODULE_CACHE["mod"]

    prep, xT_slices, cbv_slices, b_out = _host_prep(inputs)
    in_maps = []
    for c in range(NC):
        m = dict(prep)
        m["xT"] = xT_slices[c]
        m["cbv"] = cbv_slices[c]
        in_maps.append(m)

    res = run_bass_kernel_spmd(
        nc, in_maps, core_ids=list(range(NC)), trace=trace, **kwargs
    )
    outs = [np.asarray(res.results[c]["out"], dtype=np.float32) for c in range(NC)]
    full = np.concatenate(outs, axis=0).reshape(B, N, DM)
    full += b_out[None, None, :]
    return full, res


def kernel(**inputs) -> np.ndarray:
    return _run(inputs)[0]


# revision 5
# speedup vs baseline: 1.1304x; 1.1304x over previous
"""Trainium2 Bass kernel for nn_DKAModule (dynamic-kernel attention).

Decomposition (per core, data-parallel over B*n = 8192 tokens -> 1024/core
with a 10-token halo), all matmuls in bf16 (1 cycle/row on the PE vs 2 for
fp32r), PSUM accumulation in fp32:

  x_projT = W_in @ x^T                       (PE, layout (d, t), bf16)
  per head h (d_h=128 partitions, window size k_h):
    S_r    = band-matrix matmuls over token windows (PE)   [dynamic conv]
    csv_r  = S_r * cbv_r                     (DVE; cbv = alpha*V[r,d]*c_r[n]
                                              precomputed on host, bf16)
    o_h    = sum_r csv_r                     (DVE add tree, or PE ident-diag)
           + static conv: head 7 on PE diag matmuls, heads 0-6 on DVE as
             tensor_scalar (4x bf16) + tensor_add (2x bf16) shift-MACs
  out     = o^T @ W_out^T                    (PE); b_out added on host

Act (scalar) engine handles every PSUM evacuation so DVE operands stay
all-SBUF bf16 (fast 2x/4x DVE modes).
"""
import sys
import types

import numpy as np
import ml_dtypes

BF = ml_dtypes.bfloat16

KS = [3, 3, 7, 7, 11, 11, 21, 21]
H, DM, DH, R, B, N = 8, 1024, 128, 4, 2, 4096
NC = 8
TPC = B * N // NC  # tokens per core
PAD = 10
TH = TPC + 2 * PAD  # 1044
S1C = TH // 3  # 348, stage-1 chunk width

PE_STATIC_HEADS = (7,)  # static conv + r-sum on PE (others: DVE)
HEAD_ORDER = (7, 6, 0, 1, 4, 2, 5, 3)

_MODULE_CACHE = {}


def _install_ntff_hook_shim():
    """This image's antenv lacks axon_hooks; provide it so profiling works."""
    if "antenv.axon_hooks" in sys.modules:
        return
    try:
        from trn_agent_boot.trn_boot import _ntff_profile_via_ctypes

        hook = _ntff_profile_via_ctypes("/opt/axon/libaxon_pjrt.so")
    except Exception:
        hook = None
    mod = types.ModuleType("antenv.axon_hooks")
    mod.get_axon_ntff_profile_hook = lambda: hook
    mod.set_axon_ntff_profile_hook = lambda h: None
    sys.modules["antenv.axon_hooks"] = mod


def _split_multi_waits(nc, mybir):
    """walrus codegen allows a single sync-wait per instruction; hoist
    extras onto a chain of single-wait NoOps on the same engine."""
    for f in nc.m.functions:
        for blk in f.blocks:
            new_insts = []
            for inst in blk.instructions:
                si = getattr(inst, "sync_info", None)
                ow = list(si.on_wait) if si and si.on_wait else []
                if len(ow) >= 2:
                    for i, w in enumerate(ow[:-1]):
                        new_insts.append(
                            mybir.InstNoOp(
                                name=f"{inst.name}-wn{i}",
                                ins=[],
                                outs=[],
                                engine=inst.engine,
                                sync_info=mybir.SyncInfo(on_wait=[w], on_update=[]),
                            )
                        )
                    inst.sync_info = mybir.SyncInfo(
                        on_wait=[ow[-1]],
                        on_update=list(si.on_update) if si.on_update else [],
                    )
                new_insts.append(inst)
            blk.instructions = new_insts


def _window_params(h):
    k = KS[h]
    p = k // 2
    W = 128 - 2 * p
    nw = -(-TPC // W)
    return k, p, W, nw


def _build_module():
    import concourse.bass as bass
    import concourse.tile as tile
    from concourse import mybir

    f32 = mybir.dt.float32
    bf16 = mybir.dt.bfloat16
    MUL = mybir.AluOpType.mult

    nc = bass.Bass(trn_type="TRN2")

    # ---- DRAM I/O ----
    xT_d = nc.dram_tensor("xT", [DM, TH], bf16, kind="ExternalInput")
    w_inT_d = nc.dram_tensor("w_inT", [DM, DM], bf16, kind="ExternalInput")
    w_outT_d = nc.dram_tensor("w_outT", [DM, DM], bf16, kind="ExternalInput")
    cbv_d = nc.dram_tensor("cbv", [128, H * R * TPC], bf16, kind="ExternalInput")
    band_d = []
    for h in range(H):
        k, p, W, nw = _window_params(h)
        band_d.append(
            nc.dram_tensor(f"band{h}", [128, R * W], bf16, kind="ExternalInput")
        )
    gd_d = {
        h: nc.dram_tensor(f"gdiag{h}", [DH, KS[h] * DH], bf16, kind="ExternalInput")
        for h in PE_STATIC_HEADS
    }
    gvec_d = nc.dram_tensor("gvec", [DH, H * 21], f32, kind="ExternalInput")
    b_in_d = nc.dram_tensor("b_in", [DM, 1], f32, kind="ExternalInput")
    ident_d = nc.dram_tensor("ident", [128, 128], bf16, kind="ExternalInput")
    out_d = nc.dram_tensor("out", [TPC, DM], bf16, kind="ExternalOutput")

    with tile.TileContext(nc) as tc:
        with tc.tile_pool(name="const", bufs=1) as pc:
            xp_sb = [pc.tile([DH, TH], bf16, name=f"xp{m}") for m in range(H)]
            o_sb = [pc.tile([DH, TPC], bf16, name=f"o{h}") for h in range(H)]
            ident_sb = pc.tile([128, 128], bf16, name="ident_sb")
            b_in_sb = pc.tile([128, H], f32, name="b_in_sb")
            gvec_sb = pc.tile([DH, H * 21], f32, name="gvec_sb")
            nc.gpsimd.dma_start(out=ident_sb, in_=ident_d[:, :])
            nc.gpsimd.dma_start(out=gvec_sb, in_=gvec_d[:, :])
            for m in range(H):
                nc.gpsimd.dma_start(
                    out=b_in_sb[:, m : m + 1], in_=b_in_d[m * 128 : (m + 1) * 128, :]
                )

            # ---------------- stage 1: x_projT ----------------
            with tc.tile_pool(name="s1", bufs=1) as p1, tc.tile_pool(
                name="ps1", bufs=4, space="PSUM"
            ) as pp1:
                w_sb = [p1.tile([128, DM], bf16, name=f"w_in{i}") for i in range(H)]
                xT_sb = [p1.tile([128, TH], bf16, name=f"xT{i}") for i in range(H)]
                # first m-group's operands first, then the bulk
                for i in range(H):
                    nc.sync.dma_start(
                        out=w_sb[i][:, 0:128],
                        in_=w_inT_d[i * 128 : (i + 1) * 128, 0:128],
                    )
                    nc.gpsimd.dma_start(
                        out=xT_sb[i][:, 0:S1C],
                        in_=xT_d[i * 128 : (i + 1) * 128, 0:S1C],
                    )
                for i in range(H):
                    nc.sync.dma_start(
                        out=w_sb[i][:, 128:DM],
                        in_=w_inT_d[i * 128 : (i + 1) * 128, 128:DM],
                    )
                    nc.sync.dma_start(
                        out=xT_sb[i][:, S1C:TH],
                        in_=xT_d[i * 128 : (i + 1) * 128, S1C:TH],
                    )

                for ci in range(3):
                    c0 = ci * S1C
                    for m in HEAD_ORDER:
                        ps = pp1.tile([128, S1C], f32, name="ps_xp", tag="ps_xp")
                        for i in range(H):
                            nc.tensor.matmul(
                                ps,
                                w_sb[i][:, m * 128 : (m + 1) * 128],
                                xT_sb[i][:, c0 : c0 + S1C],
                                start=(i == 0),
                                stop=(i == H - 1),
                            )
                        nc.scalar.activation(
                            out=xp_sb[m][:, c0 : c0 + S1C],
                            in_=ps,
                            func=mybir.ActivationFunctionType.Identity,
                            bias=b_in_sb[:, m : m + 1],
                            scale=1.0,
                        )

            # out-projection weights prefetched mid-stage-2 (see below)
            p4ctx = tc.tile_pool(name="s4w", bufs=1)
            p4 = p4ctx.__enter__()
            wo_sb = [p4.tile([128, DM], bf16, name=f"w_out{i}") for i in range(H)]

            # ---------------- stage 2: per-head convs ----------------
            with tc.tile_pool(name="s2b", bufs=2) as p2b, tc.tile_pool(
                name="s2cbv", bufs=3
            ) as p2c, tc.tile_pool(name="s2csv", bufs=3) as p2v, tc.tile_pool(
                name="s2s", bufs=3
            ) as p2s, tc.tile_pool(name="s2x", bufs=4) as p2x, tc.tile_pool(
                name="s2acc", bufs=2
            ) as p2a, tc.tile_pool(name="s2t", bufs=3) as p2t, tc.tile_pool(
                name="ps2tp", bufs=3, space="PSUM"
            ) as pptp, tc.tile_pool(
                name="ps2s", bufs=3, space="PSUM"
            ) as ppss, tc.tile_pool(
                name="ps2o", bufs=2, space="PSUM"
            ) as ppso:
                cbv4 = cbv_d.rearrange("p (hh r t) -> p hh r t", hh=H, r=R)
                gd_sb = {}
                for h in PE_STATIC_HEADS:
                    g = pc.tile([DH, KS[h] * DH], bf16, name=f"gd{h}")
                    nc.sync.dma_start(out=g, in_=gd_d[h][:, :])
                    gd_sb[h] = g

                csv3_of = {}

                # prefetch the first heads' coefficient tiles before stage-2
                # compute begins (sync queue: less contended than gpsimd)
                cbv_sb_of = {}
                for h in HEAD_ORDER[:3]:
                    cbv_sb = p2c.tile([DH, R * TPC], bf16, name=f"cbv{h}", tag="cbv")
                    nc.sync.dma_start(
                        out=cbv_sb.rearrange("p (r t) -> p r t", r=R),
                        in_=cbv4[:, h, :, :],
                    )
                    cbv_sb_of[h] = cbv_sb

                def stage2a(h):
                    k, p, W, nw = _window_params(h)
                    xph = xp_sb[h]
                    band_sb = p2b.tile([128, R * W], bf16, name=f"band{h}", tag="band")
                    nc.sync.dma_start(out=band_sb, in_=band_d[h][:, :])
                    band3 = band_sb.rearrange("p (r w) -> p r w", r=R)
                    if h in cbv_sb_of:
                        cbv_sb = cbv_sb_of.pop(h)
                    else:
                        cbv_sb = p2c.tile([DH, R * TPC], bf16, name=f"cbv{h}", tag="cbv")
                        nc.sync.dma_start(
                            out=cbv_sb.rearrange("p (r t) -> p r t", r=R),
                            in_=cbv4[:, h, :, :],
                        )
                    cbv3 = cbv_sb.rearrange("p (r t) -> p r t", r=R)
                    csv_sb = p2v.tile([DH, R * TPC], bf16, name=f"csv{h}", tag="csv")
                    csv3 = csv_sb.rearrange("p (r t) -> p r t", r=R)
                    csv3_of[h] = csv3
                    for b in range(nw):
                        off = PAD - p + b * W
                        cnt = min(128, TH - off)
                        n_out = min(W, TPC - b * W)
                        tp = pptp.tile([128, 128], bf16, name="tp", tag="tp")
                        nc.tensor.transpose(
                            tp[:cnt, :], xph[:, off : off + cnt], ident_sb
                        )
                        xtd = p2x.tile([128, 128], bf16, name="xtd", tag="xtd")
                        nc.scalar.copy(xtd[:cnt, :], tp[:cnt, :])
                        ps_s = ppss.tile([128, R * W], f32, name="ps_s", tag="ps_s")
                        s_sb = p2s.tile([128, R * W], bf16, name="s_sb", tag="s_sb")
                        if n_out == W:
                            # contiguous 2D moving/out APs (faster PE streaming)
                            nc.tensor.matmul(
                                ps_s, xtd[:cnt, :], band_sb[:cnt, :],
                                start=True, stop=True,
                            )
                            nc.scalar.copy(s_sb, ps_s)
                        else:
                            nc.tensor.matmul(
                                ps_s[:, : R * n_out],
                                xtd[:cnt, :],
                                band3[:cnt, :, :n_out],
                                start=True,
                                stop=True,
                            )
                            nc.scalar.copy(s_sb[:, : R * n_out], ps_s[:, : R * n_out])
                        nc.vector.tensor_mul(
                            csv3[:, :, b * W : b * W + n_out],
                            s_sb[:, : R * n_out].rearrange("p (r w) -> p r w", r=R),
                            cbv3[:, :, b * W : b * W + n_out],
                        )

                # stage-2b work is emitted in token-chunks so the DVE stream
                # never blocks a whole head's csv pipeline for long
                def stage2b_chunks(h):
                    k, p, W, nw = _window_params(h)
                    xph = xp_sb[h]
                    if h in PE_STATIC_HEADS:
                        def pe_chunk(c0):
                            csv3 = csv3_of[h]
                            ps_o = ppso.tile([128, 512], f32, name="ps_o", tag="ps_o")
                            nmm = R + k
                            idx = 0
                            for rr in range(R):
                                nc.tensor.matmul(
                                    ps_o,
                                    ident_sb,
                                    csv3[:, rr, c0 : c0 + 512],
                                    start=(idx == 0),
                                    stop=(idx == nmm - 1),
                                )
                                idx += 1
                            for j in range(k):
                                o0 = c0 + j - p + PAD
                                nc.tensor.matmul(
                                    ps_o,
                                    gd_sb[h][:, j * DH : (j + 1) * DH],
                                    xph[:, o0 : o0 + 512],
                                    start=(idx == 0),
                                    stop=(idx == nmm - 1),
                                )
                                idx += 1
                            nc.scalar.copy(o_sb[h][:, c0 : c0 + 512], ps_o)
                        return [lambda c0=c0: pe_chunk(c0) for c0 in (0, 512)]
                    # DVE head: static shift-MACs + r-sum, in halves
                    sacc = p2a.tile([DH, TPC], bf16, name="sacc", tag="sacc")
                    HC = TPC // 2

                    def dve_chunk(c0):
                        csv3 = csv3_of[h]
                        sl = slice(c0, c0 + HC)
                        for j in range(k):
                            sh = PAD + j - p + c0
                            gcol = gvec_sb[:, h * 21 + j : h * 21 + j + 1]
                            if j == 0:
                                nc.vector.tensor_scalar(
                                    out=sacc[:, sl],
                                    in0=xph[:, sh : sh + HC],
                                    scalar1=gcol,
                                    scalar2=None,
                                    op0=MUL,
                                )
                            else:
                                tmp = p2t.tile([DH, HC], bf16, name="tmp", tag="tmp")
                                nc.vector.tensor_scalar(
                                    out=tmp,
                                    in0=xph[:, sh : sh + HC],
                                    scalar1=gcol,
                                    scalar2=None,
                                    op0=MUL,
                                )
                                nc.vector.tensor_add(sacc[:, sl], sacc[:, sl], tmp)
                        t01 = p2t.tile([DH, HC], bf16, name="t01", tag="tmp")
                        nc.vector.tensor_add(t01, csv3[:, 0, sl], csv3[:, 1, sl])
                        t23 = p2t.tile([DH, HC], bf16, name="t23", tag="tmp")
                        nc.vector.tensor_add(t23, csv3[:, 2, sl], csv3[:, 3, sl])
                        nc.vector.tensor_add(t01, t01, t23)
                        nc.vector.tensor_add(o_sb[h][:, sl], t01, sacc[:, sl])

                    return [lambda c0=c0: dve_chunk(c0) for c0 in (0, HC)]

                pending = []
                for hi, h in enumerate(HEAD_ORDER):
                    stage2a(h)
                    if hi == 3:  # mid-stage-2: prefetch stage-4 weights
                        for i in range(H):
                            nc.gpsimd.dma_start(
                                out=wo_sb[i],
                                in_=w_outT_d[i * 128 : (i + 1) * 128, :],
                            )
                    if hi >= 1:
                        pending.extend(stage2b_chunks(HEAD_ORDER[hi - 1]))
                        # drain up to 2 chunks per head slot
                        for _ in range(2):
                            if pending:
                                pending.pop(0)()
                pending.extend(stage2b_chunks(HEAD_ORDER[-1]))
                for fn in pending:
                    fn()

            # ---------------- stage 4: out projection ----------------
            with tc.tile_pool(name="s4o", bufs=3) as p4o, tc.tile_pool(
                name="ps4", bufs=4, space="PSUM"
            ) as pp4:
                for t in range(TPC // 128):
                    ot = p4o.tile([128, DM], bf16, name="out_sb", tag="out_sb")
                    for e0 in (0, 512):
                        ps = pp4.tile([128, 512], f32, name="ps_out", tag="ps_out")
                        for i in range(H):
                            nc.tensor.matmul(
                                ps,
                                o_sb[i][:, t * 128 : (t + 1) * 128],
                                wo_sb[i][:, e0 : e0 + 512],
                                start=(i == 0),
                                stop=(i == H - 1),
                            )
                        nc.scalar.copy(ot[:, e0 : e0 + 512], ps)
                        eng = nc.gpsimd if (t % 2 == 0) else nc.sync
                        eng.dma_start(
                            out=out_d[t * 128 : (t + 1) * 128, e0 : e0 + 512],
                            in_=ot[:, e0 : e0 + 512],
                        )
            p4ctx.__exit__(None, None, None)

    _split_multi_waits(nc, mybir)
    return nc


def _host_prep(inputs):
    x = np.ascontiguousarray(np.asarray(inputs["x"], dtype=np.float32))
    W_in = np.asarray(inputs["W_in"], dtype=np.float32)
    b_in = np.asarray(inputs["b_in"], dtype=np.float32)
    W_out = np.asarray(inputs["W_out"], dtype=np.float32)
    b_out = np.asarray(inputs["b_out"], dtype=np.float32)
    Wc = np.asarray(inputs["Wc"], dtype=np.float32)
    A = np.asarray(inputs["A"], dtype=np.float32)
    V = np.asarray(inputs["V"], dtype=np.float32)
    base = np.asarray(inputs["base"], dtype=np.float32)
    alphas = np.asarray(inputs["alphas"], dtype=np.float32)

    alpha = 1.0 / (1.0 + np.exp(-alphas))

    W_inT = np.ascontiguousarray(W_in.T)
    W_outT = np.ascontiguousarray(W_out.T)
    # c = x_proj_head @ Wc[h] folded to raw x: x @ Wc_aug + c_bias (from b_in)
    Wc_aug = np.zeros((DM, H * R), dtype=np.float32)
    c_bias = np.zeros((H * R,), dtype=np.float32)
    for h in range(H):
        Wc_aug[:, R * h : R * h + R] = W_inT[:, h * DH : (h + 1) * DH] @ Wc[h]
        c_bias[R * h : R * h + R] = b_in[h * DH : (h + 1) * DH] @ Wc[h]

    prep = {
        "w_inT": W_inT.astype(BF),
        "w_outT": W_outT.astype(BF),
        "b_in": b_in.reshape(DM, 1).copy(),
        "ident": np.eye(128, dtype=BF),
    }

    for h in range(H):
        k, p, W, nw = _window_params(h)
        t_in = np.arange(128)[:, None]
        t_out = np.arange(W)[None, :]
        delta = t_in - t_out
        mask = (delta >= 0) & (delta < k)
        band = np.zeros((128, R, W), dtype=np.float32)
        dc = np.clip(delta, 0, k - 1)
        for rr in range(R):
            band[:, rr, :] = np.where(mask, A[h, rr][dc], 0.0)
        prep[f"band{h}"] = band.reshape(128, R * W).astype(BF)

    for h in PE_STATIC_HEADS:
        k = KS[h]
        gd = np.zeros((DH, k, DH), dtype=np.float32)
        g = (1.0 - alpha[h]) * base[h, :k]  # (k, DH)
        dd = np.arange(DH)
        gd[dd, :, dd] = g.T[dd]
        prep[f"gdiag{h}"] = gd.reshape(DH, k * DH).astype(BF)

    gvec = np.zeros((DH, H, 21), dtype=np.float32)
    for h in range(H):
        k = KS[h]
        gvec[:, h, :k] = ((1.0 - alpha[h]) * base[h, :k]).T
    prep["gvec"] = gvec.reshape(DH, H * 21).copy()

    # per-core transposed x slices with halo + zero padding, plus host-side
    # coefficient computation with alpha*V folded in:
    # cbv[d, h, r, t] = alpha[h] * V[h,r,d] * c[t, h, r]
    xT_slices = []
    cbv_slices = []
    per_b = NC // B
    for c in range(NC):
        bb = c // per_b
        s = (c % per_b) * TPC
        sl = np.zeros((TH, DM), dtype=np.float32)
        lo, hi = s - PAD, s + TPC + PAD
        clo, chi = max(lo, 0), min(hi, N)
        sl[clo - lo : chi - lo] = x[bb, clo:chi]
        xT_slices.append(np.ascontiguousarray(sl.T).astype(BF))
        cc = sl[PAD : PAD + TPC] @ Wc_aug + c_bias[None, :]  # (TPC, H*R)
        cc3 = cc.reshape(TPC, H, R)
        cbv = np.empty((128, H, R, TPC), dtype=np.float32)
        for h in range(H):
            for rr in range(R):
                cbv[:, h, rr, :] = alpha[h] * np.outer(V[h, rr], cc3[:, h, rr])
        cbv_slices.append(
            np.ascontiguousarray(cbv.reshape(128, H * R * TPC)).astype(BF)
        )
    return prep, xT_slices, cbv_slices, b_out


def _run(inputs, trace=False, **kwargs):
    _install_ntff_hook_shim()
    from concourse.bass_utils import run_bass_kernel_spmd

    if "mod" not in _MODULE_CACHE:
        _MODULE_CACHE["mod"] = _build_module()
    nc = _MODULE_CACHE["mod"]

    prep, xT_slices, cbv_slices, b_out = _host_prep(inputs)
    in_maps = []
    for c in range(NC):
        m = dict(prep)
        m["xT"] = xT_slices[c]
        m["cbv"] = cbv_slices[c]
        in_maps.append(m)

    res = run_bass_kernel_spmd(
        nc, in_maps, core_ids=list(range(NC)), trace=trace, **kwargs
    )
    outs = [np.asarray(res.results[c]["out"], dtype=np.float32) for c in range(NC)]
    full = np.concatenate(outs, axis=0).reshape(B, N, DM)
    full += b_out[None, None, :]
    return full, res


def kernel(**inputs) -> np.ndarray:
    return _run(inputs)[0]
